# revision 28
# baseline (speedup 1.0000x reference)
"""Trainium2 Bass kernel for the GNN message-passing autoencoder problem.

Strategy (8 NeuronCores, SPMD):
  - Nodes sharded 1024/core. Message passing lowered to dense fp8 DoubleRow
    matmuls against per-core column shards of A^T (normalization folded in
    on host). h is carried in fp8 through the layers and AllGathers.
  - Per layer: A-matmul in two 512-dst halves (4 PSUM banks), W-matmul in
    bf16 (2 banks), BN stats AllReduce (4KB), BN+PReLU fused scalar
    activations, PE transposes to node-major fp8, AllGather.
  - The two chains are software-pipelined with explicit emission order
    [M1-n0][T2][M1-rest][M2-n0][T1][M2-rest] so collectives and DMA hide
    under the opposite chain's matmul phases.
  - loss2 via the Gram identity: sum((H H^T - M)^2) = ||H^T H||_F^2
    - 2*sum_i H_i . (M H)_i + ||M||_F^2. Each core computes G-partials,
    an AllReduce of G, a row-shard of P = M @ H (fp8 DoubleRow), and
    streams ||M||^2 squares on the scalar engine under the P matmuls.
"""

import os
import sys

for _p in ("/opt/trn_rl_repo", "/opt/pypackages"):
    if _p not in sys.path:
        sys.path.append(_p)

import numpy as np
import ml_dtypes

import concourse.bass as bass
import concourse.mybir as mybir
import concourse.tile as tile
from concourse import bacc
from concourse.bass_utils import run_bass_kernel_spmd
from concourse.masks import make_identity

F8 = mybir.dt.float8e4
BF16 = mybir.dt.bfloat16
F32 = mybir.dt.float32
AF = mybir.ActivationFunctionType
ALU = mybir.AluOpType
AX = mybir.AxisListType
DR = mybir.MatmulPerfMode.DoubleRow

N = 8192
F = 512
NCORES = 8
SH = N // NCORES          # 1024 nodes per core shard
NB = N // 128             # 64 node k-tiles
SB = SH // 128            # 8 node blocks per shard
FB = F // 128             # 4 feature blocks
ACK = 8                   # A k-tiles per DMA chunk
MCK = 16                  # mtb k-tiles per DMA chunk

# layer-instance parameter rows: enc0 enc1 dec1_0 dec1_1 dec2_0 dec2_1
LI_ENC0, LI_ENC1, LI_D10, LI_D11, LI_D20, LI_D21 = range(6)


def _art_dma(nc, sb, st, c, li, n, cc):
    """Issue the DMA for one A^T chunk into the rotating art pool."""
    aT = st["aT"][(c, li)]
    art = sb.tile([128, ACK, 512], F8, tag=f"art{c}", bufs=4, name="art")
    nc.gpsimd.dma_start(art[:], aT[n, :, cc * ACK:(cc + 1) * ACK, :])
    return art


def _emit_A_chunk(nc, st, c, mps, art, cc, n):
    """The 16 DoubleRow matmuls for one 8-ktile art chunk."""
    half = st["hsb"][c][cc * ACK // (NB // 2)]
    for k2 in range(0, ACK, 2):
        k = cc * ACK + k2
        kk = k % (NB // 2)
        for m in range(FB):
            nc.tensor.matmul(
                mps[m][:],
                half[:, kk:kk + 2, m * 128:(m + 1) * 128],
                art[:, k2:k2 + 2, :],
                start=(k == 0), stop=(k == NB - 2), perf_mode=DR)


def _emit_A_half(nc, sb, ps, st, c, li, n, mps, pre=(), skip_chunks=0):
    """A-matmul for dst-half n: accumulate mT[f, n*512:(n+1)*512] over all
    64 src k-tiles using fp8 DoubleRow. mps: 4 PSUM tiles (one per f-block).
    pre: art tiles whose DMA was already issued (prefetch hoisting);
    skip_chunks: chunks already emitted elsewhere (transpose fill)."""
    for cc in range(skip_chunks, NB // ACK):
        if cc < len(pre):
            art = pre[cc]
        else:
            art = _art_dma(nc, sb, st, c, li, n, cc)
        _emit_A_chunk(nc, st, c, mps, art, cc, n)


def _emit_evict_n0(nc, sb, ps, st, c, mps0):
    """Evict A-half-0 PSUM tiles to mt (frees mmps slots for the n1 half)."""
    mt = st["mt"][c]
    for m in range(FB):
        dst = mt[:, m, 0:512]
        if m % 2 == 0:
            nc.vector.tensor_copy(dst, mps0[m][:])
        else:
            nc.scalar.copy(dst, mps0[m][:])


def _emit_M_rest(nc, sb, ps, g, st, c, l, li, mps0, pre=(), hoist=None):
    """Second part of the matmul phase: evict n0, A-matmul n1, evict n1,
    W-matmul both halves, bias+PReLU eviction, BN stats, AR trigger."""
    mt = st["mt"][c]
    zt = st["zt"][c]
    stats = sb.tile([128, 16], F32, tag=f"stats{c}", bufs=1, name="stats")

    wsb = sb.tile([128, FB, F], F8, tag="wsb", bufs=2, name="wsb")
    nc.gpsimd.dma_start(wsb[:], g["w_all"][li].rearrange("(t p) fo -> p t fo", p=128))

    def emit_W(n):
        # W-matmul (fp8 DoubleRow) + bias/PReLU eviction + BN stats for half n
        for m in range(FB):
            zps = ps.tile([128, 512], F32, tag="mmps", bufs=4, name="zps")
            for kb in range(0, FB, 2):
                nc.tensor.matmul(
                    zps[:],
                    wsb[:, kb:kb + 2, m * 128:(m + 1) * 128],
                    mt[:, kb:kb + 2, n * 512:(n + 1) * 512],
                    start=(kb == 0), stop=(kb == FB - 2), perf_mode=DR)
            zslc = zt[:, m, n * 512:(n + 1) * 512]
            nc.scalar.activation(
                zslc, zps[:], AF.Prelu,
                bias=g["b_sb"][:, li, m:m + 1], scale=1.0,
                alpha=g["al_sb"][:, 2 * li:2 * li + 1])
            col = n * 8 + 2 * m
            nc.vector.reduce_sum(stats[:, col:col + 1], zslc, axis=AX.X)
            scr = sb.tile([128, 512], F32, tag="scr", bufs=4, name="scr")
            nc.scalar.activation(scr[:], zslc, AF.Square,
                                 accum_out=stats[:, col + 1:col + 2])

    _emit_evict_n0(nc, sb, ps, st, c, mps0)
    emit_W(0)      # runs off mt-n0 while its stats overlap the n1 matmuls
    mps1 = [ps.tile([128, 512], F32, tag="mmps", bufs=4, name="mmps")
            for _ in range(FB)]
    _emit_A_half(nc, sb, ps, st, c, li, 1, mps1, pre=pre)
    for m in range(FB):
        dst = mt[:, m, 512:1024]
        if m % 2 == 0:
            nc.vector.tensor_copy(dst, mps1[m][:])
        else:
            nc.scalar.copy(dst, mps1[m][:])
    emit_W(1)

    # pre-sum the half stats so the AllReduce payload is 4KB
    ssum = sb.tile([128, 8], F32, tag=f"ssum{c}", bufs=1, name="ssum")
    nc.vector.tensor_add(ssum[:], stats[:, 0:8], stats[:, 8:16])
    ar_in, ar_out = st["ar"][(c, l)]
    nc.scalar.dma_start(ar_in[:], ssum[:])
    hoisted = hoist() if hoist else None
    nc.gpsimd.collective_compute(
        "AllReduce", ALU.add, replica_groups=[list(range(NCORES))],
        ins=[ar_in[:]], outs=[ar_out[:]])
    return hoisted


def _emit_bn_apply(nc, sb, g, st, c, l, li):
    """BN finalize from the AllReduced stats and fused BN+PReLU apply."""
    zt = st["zt"][c]
    ar_in, ar_out = st["ar"][(c, l)]
    gstats = sb.tile([128, 8], F32, tag=f"gstats{c}", bufs=1, name="gstats")
    nc.sync.dma_start(gstats[:], ar_out[:])
    mean = sb.tile([128, FB], F32, tag=f"mean{c}", name="mean")
    var = sb.tile([128, FB], F32, tag=f"var{c}", name="var")
    sN = sb.tile([128, FB], F32, tag=f"sN{c}", name="sN")
    tN = sb.tile([128, FB], F32, tag=f"tN{c}", name="tN")
    m2 = sb.tile([128, FB], F32, tag=f"m2{c}", name="m2")
    nc.scalar.mul(mean[:], gstats[:, 0:8:2], 1.0 / N)
    nc.scalar.mul(var[:], gstats[:, 1:8:2], 1.0 / N)   # E[x^2]
    nc.vector.tensor_mul(m2[:], mean[:], mean[:])
    nc.vector.tensor_sub(var[:], var[:], m2[:])
    nc.scalar.activation(sN[:], var[:], AF.Sqrt, bias=g["epsb"][:])
    nc.vector.reciprocal(sN[:], sN[:])
    nc.vector.tensor_mul(sN[:], sN[:], g["g_sb"][:, li, :])
    nc.vector.tensor_mul(m2[:], mean[:], sN[:])
    nc.vector.tensor_sub(tN[:], g["bb_sb"][:, li, :], m2[:])
    for m in range(FB):
        nc.scalar.activation(
            zt[:, m, :], zt[:, m, :], AF.Prelu,
            bias=tN[:, m:m + 1], scale=sN[:, m:m + 1],
            alpha=g["al_sb"][:, 2 * li + 1:2 * li + 2])


def _emit_T(nc, sb, ps, g, st, c, l, li, hoist=None):
    """Layer tail: BN finalize+apply, transpose to node-major fp8,
    AllGather trigger, and the next layer's h load."""
    hoisted = hoist() if hoist else None
    _emit_bn_apply(nc, sb, g, st, c, l, li)
    zt = st["zt"][c]
    hsb = st["hsb"][c]
    hnm = sb.tile([128, SB, F], F8, tag=f"hnm{c}", bufs=1, name="hnm")
    ag_in, ag_out = st["ag"][(c, l)]
    agf = ag_in.rearrange("p (t f) -> p t f", f=F)
    for t in range(SB):
        for m in range(FB):
            tp = ps.tile([128, 128], BF16, tag="tp", bufs=4, name="tp")
            nc.tensor.transpose(tp[:], zt[:, m, t * 128:(t + 1) * 128],
                                g["ident"][:])
            nc.vector.tensor_copy(hnm[:, t, m * 128:(m + 1) * 128], tp[:])
        if t in (3, SB - 1):
            t0 = 0 if t == 3 else 4
            nc.gpsimd.dma_start(agf[:, t0:t + 1, :], hnm[:, t0:t + 1, :])
    nc.gpsimd.collective_compute(
        "AllGather", ALU.bypass, replica_groups=[list(range(NCORES))],
        ins=[ag_in[:]], outs=[ag_out[:]])
    agp = ag_out.rearrange("(b c p) (t f) -> p b c t f", p=128, c=4, f=F)
    for b in range(2):
        nc.sync.dma_start(
            hsb[b].rearrange("p (c t) f -> p c t f", c=4), agp[:, b, :, :, :])
    if (c, l) in st["dbg"]:
        nc.sync.dma_start(st["dbg"][(c, l)].rearrange("(t p) f -> p t f", p=128),
                          hnm[:])
    return hoisted


def _emit_m2_pass(nc, sb, st, blocks):
    """||M||^2 partial sums for the given row blocks. Self-paced on the
    scalar engine (its own DMA queue) so it cannot stall the art stream."""
    m2c = st["m2c"]
    for b in blocks:
        for cc in range(NB // MCK):
            mtc = sb.tile([128, MCK, 128], F8, tag="mtc2", bufs=2, name="mtc2")
            nc.scalar.dma_start(mtc[:], st["mtb"][b, :, cc * MCK:(cc + 1) * MCK, :])
            flat = mtc.rearrange("p t i -> p (t i)")
            for q in range(4):
                col = (b * (NB // MCK) + cc) * 4 + q
                scr = sb.tile([128, F], F32, tag="scr", bufs=4, name="scr")
                nc.scalar.activation(scr[:], flat[:, q * 512:(q + 1) * 512],
                                     AF.Square, accum_out=m2c[:, col:col + 1])


def _emit_tail1(nc, sb, ps, g, st):
    """Chain-1 tail: BN apply, transpose, sce-loss partial over the shard."""
    _emit_bn_apply(nc, sb, g, st, 1, 3, LI_D11)
    zt = st["zt"][1]
    h1nm = sb.tile([128, SB, F], BF16, tag="h1nm", bufs=1, name="h1nm")
    for t in range(SB):
        for m in range(FB):
            tp = ps.tile([128, 128], BF16, tag="tp", bufs=4, name="tp")
            nc.tensor.transpose(tp[:], zt[:, m, t * 128:(t + 1) * 128],
                                g["ident"][:])
            nc.vector.tensor_copy(h1nm[:, t, m * 128:(m + 1) * 128], tp[:])
    attr_sb = sb.tile([128, SB, F], BF16, tag="attr_sb", bufs=1, name="attr_sb")
    nc.gpsimd.dma_start(attr_sb[:], st["attr_sh"][:])
    dot = sb.tile([128, SB], F32, tag="dot", name="dot")
    n1 = sb.tile([128, SB], F32, tag="n1", name="n1")
    n2 = sb.tile([128, SB], F32, tag="n2", name="n2")
    for t in range(SB):
        scr = sb.tile([128, F], F32, tag="scr", bufs=4, name="scr")
        nc.vector.tensor_mul(scr[:], h1nm[:, t, :], attr_sb[:, t, :])
        nc.vector.reduce_sum(dot[:, t:t + 1], scr[:], axis=AX.X)
        scr2 = sb.tile([128, F], F32, tag="scr", bufs=4, name="scr")
        nc.scalar.activation(scr2[:], h1nm[:, t, :], AF.Square,
                             accum_out=n1[:, t:t + 1])
        scr3 = sb.tile([128, F], F32, tag="scr", bufs=4, name="scr")
        nc.scalar.activation(scr3[:], attr_sb[:, t, :], AF.Square,
                             accum_out=n2[:, t:t + 1])
    # u = 1 - dot/sqrt(n1*n2); l1p = sum(u^3)
    p12 = sb.tile([128, SB], F32, tag="p12", name="p12")
    nc.vector.tensor_mul(p12[:], n1[:], n2[:])
    nc.scalar.activation(p12[:], p12[:], AF.Sqrt)
    nc.vector.reciprocal(p12[:], p12[:])
    nc.vector.tensor_mul(dot[:], dot[:], p12[:])
    u = sb.tile([128, SB], F32, tag="u", name="u")
    nc.scalar.activation(u[:], dot[:], AF.Copy, scale=-1.0, bias=1.0)
    u2 = sb.tile([128, SB], F32, tag="u2", name="u2")
    nc.vector.tensor_mul(u2[:], u[:], u[:])
    nc.vector.tensor_mul(u2[:], u2[:], u[:])
    l1p = sb.tile([128, 1], F32, tag="l1p", name="l1p")
    nc.vector.reduce_sum(l1p[:], u2[:], axis=AX.X)
    st["l1p"] = l1p


def _emit_tail2_head(nc, sb, ps, g, st):
    """Chain-2 tail: BN apply, l2-normalize, AllGather h2n (fp8), per-core
    Gram partial G + AllReduce, P = M_shard @ H via fp8 DoubleRow with
    fused dot and ||M||^2 accumulation, then the combined partials."""
    _emit_bn_apply(nc, sb, g, st, 2, 3, LI_D21)
    zt = st["zt"][2]
    h2n = sb.tile([128, SB, F], BF16, tag="h2n", bufs=1, name="h2n")
    for t in range(SB):
        for m in range(FB):
            tp = ps.tile([128, 128], BF16, tag="tp", bufs=4, name="tp")
            nc.tensor.transpose(tp[:], zt[:, m, t * 128:(t + 1) * 128],
                                g["ident"][:])
            nc.vector.tensor_copy(h2n[:, t, m * 128:(m + 1) * 128], tp[:])
    # l2 norms per node
    nrm = sb.tile([128, SB], F32, tag="nrm", name="nrm")
    for t in range(SB):
        scr = sb.tile([128, F], F32, tag="scr", bufs=4, name="scr")
        nc.scalar.activation(scr[:], h2n[:, t, :], AF.Square,
                             accum_out=nrm[:, t:t + 1])
    nc.scalar.activation(nrm[:], nrm[:], AF.Sqrt)
    nc.vector.tensor_scalar_max(nrm[:], nrm[:], 1e-12)
    nc.vector.reciprocal(nrm[:], nrm[:])
    for t in range(SB):
        nc.vector.tensor_scalar_mul(h2n[:, t, :], h2n[:, t, :], nrm[:, t:t + 1])
    # fp8 copy + AllGather
    hnm = sb.tile([128, SB, F], F8, tag="hnm2", bufs=1, name="hnm2")
    hf = st["hsb"][2]
    pre_mtc = []
    for cc in range(2):
        mtc = sb.tile([128, MCK, 128], F8, tag="mtc", bufs=3, name="mtc")
        nc.gpsimd.dma_start(mtc[:], st["mtb"][0, :, cc * MCK:(cc + 1) * MCK, :])
        pre_mtc.append(mtc)
    ag_in, ag_out = st["ag2"]
    agf = ag_in.rearrange("p (t f) -> p t f", f=F)
    for t in range(SB):
        nc.vector.tensor_copy(hnm[:, t, :], h2n[:, t, :])
        if t in (3, SB - 1):
            t0 = 0 if t == 3 else 4
            nc.gpsimd.dma_start(agf[:, t0:t + 1, :], hnm[:, t0:t + 1, :])
    nc.gpsimd.collective_compute(
        "AllGather", ALU.bypass, replica_groups=[list(range(NCORES))],
        ins=[ag_in[:]], outs=[ag_out[:]])
    agp = ag_out.rearrange("(b c p) (t f) -> p b c t f", p=128, c=4, f=F)
    for b in range(2):
        nc.sync.dma_start(
            hf[b].rearrange("p (c t) f -> p c t f", c=4), agp[:, b, :, :, :])
    if "h2n" in st["dbg"]:
        nc.sync.dma_start(st["dbg"]["h2n"].rearrange("(t p) f -> p t f", p=128),
                          hnm[:])

    # Gram partial G = sum_{i in shard} h2n_i h2n_i^T  -> AllReduce
    gps = [ps.tile([128, 512], F32, tag="mmps", bufs=4, name="mmps") for _ in range(FB)]
    for m in range(FB):
        for t in range(SB):
            nc.tensor.matmul(gps[m][:], h2n[:, t, m * 128:(m + 1) * 128],
                             h2n[:, t, :], start=(t == 0), stop=(t == SB - 1))
    g_sb = sb.tile([128, FB, F], F32, tag="g_sb2", bufs=1, name="g_sb2")
    for m in range(FB):
        if m % 2 == 0:
            nc.vector.tensor_copy(g_sb[:, m, :], gps[m][:])
        else:
            nc.scalar.copy(g_sb[:, m, :], gps[m][:])
    nc.sync.dma_start(st["arg_in"][:], g_sb.rearrange("p m f -> p (m f)"))
    nc.gpsimd.collective_compute(
        "ReduceScatter", ALU.add, replica_groups=[list(range(NCORES))],
        ins=[st["arg_in"][:]], outs=[st["arg_out"][:]])
    st["t2_carry"] = (h2n, pre_mtc)


def _emit_tail2_body(nc, sb, ps, g, st):
    h2n, pre_mtc = st["t2_carry"]
    hf = st["hsb"][2]
    # P = M_shard @ H row-block by row-block; fuse dot and ||M||^2
    dotc = sb.tile([128, SB], F32, tag="dotc", name="dotc")
    m2c = st["m2c"]
    for b in range(SB):
        pps = ps.tile([128, 512], F32, tag="mmps", bufs=4, name="pps")
        for cc in range(NB // MCK):
            if b == 0 and cc < len(pre_mtc):
                mtc = pre_mtc[cc]
            else:
                mtc = sb.tile([128, MCK, 128], F8, tag="mtc", bufs=3, name="mtc")
                nc.gpsimd.dma_start(mtc[:],
                                    st["mtb"][b, :, cc * MCK:(cc + 1) * MCK, :])
            for k2 in range(0, MCK, 2):
                k = cc * MCK + k2
                hfh = hf[k // (NB // 2)]
                kk = k % (NB // 2)
                nc.tensor.matmul(
                    pps[:], mtc[:, k2:k2 + 2, :], hfh[:, kk:kk + 2, :],
                    start=(k == 0), stop=(k == NB - 2), perf_mode=DR)
        scr2 = sb.tile([128, F], F32, tag="scr", bufs=4, name="scr")
        nc.vector.tensor_mul(scr2[:], pps[:], h2n[:, b, :])
        nc.vector.reduce_sum(dotc[:, b:b + 1], scr2[:], axis=AX.X)

    # ||G||^2 partial from this core's ReduceScatter slice of the Gram sum
    garr = sb.tile([128 // NCORES, FB * F], F32, tag="attr_sb", bufs=1,
                   name="garr")
    nc.sync.dma_start(garr[:], st["arg_out"][:])
    g2 = sb.tile([128, FB], F32, tag="g2", name="g2")
    nc.vector.memset(g2[:], 0.0)
    for q in range(FB):
        scr3 = sb.tile([128, F], F32, tag="scr", bufs=4, name="scr")
        nc.scalar.activation(scr3[0:128 // NCORES, :],
                             garr[:, q * F:(q + 1) * F],
                             AF.Square, accum_out=g2[0:128 // NCORES, q:q + 1])

    # partials: [l1p, sum(dot), sum(m2), g2] reduced across partitions
    pl = sb.tile([128, 4], F32, tag="pl", name="pl")
    nc.vector.tensor_copy(pl[:, 0:1], st["l1p"][:])
    nc.vector.reduce_sum(pl[:, 1:2], dotc[:], axis=AX.X)
    nc.vector.reduce_sum(pl[:, 2:3], m2c[:], axis=AX.X)
    nc.vector.reduce_sum(pl[:, 3:4], g2[:], axis=AX.X)
    ones = sb.tile([128, 1], F32, tag="ones", name="ones")
    nc.vector.memset(ones[:], 1.0)
    pp = ps.tile([128, 512], F32, tag="mmps", bufs=4, name="pp")
    nc.tensor.matmul(pp[0:4, 0:1], pl[:], ones[:], start=True, stop=True)
    out_sb = sb.tile([4, 1], F32, tag="out_sb", name="out_sb")
    nc.scalar.copy(out_sb[:], pp[0:4, 0:1])
    nc.sync.dma_start(st["partials"][:], out_sb[:])


def build_nc():
    nc = bacc.Bacc("TRN2", target_bir_lowering=False, debug=False,
                   num_devices=NCORES)

    ins = {}
    def di(name, shape, dt):
        ins[name] = nc.dram_tensor(name, shape, dt, kind="ExternalInput")
        return ins[name]

    h1_0 = di("h1_0", [128, NB, F], F8)       # masked x, partition-major fp8
    h2_0 = di("h2_0", [128, NB, F], F8)       # attr
    a1n = di("a1n", [2, 128, NB, 512], F8)    # enc-normalized A1^T shard
    a1p = di("a1p", [2, 128, NB, 512], F8)    # plain A1^T shard
    a2n = di("a2n", [2, 128, NB, 512], F8)
    a2p = di("a2p", [2, 128, NB, 512], F8)
    w_all = di("w_all", [6, F, F], F8)
    b_all = di("b_all", [6, F], F32)
    g_all = di("g_all", [6, F], F32)
    bb_all = di("bb_all", [6, F], F32)
    al_all = di("al_all", [1, 12], F32)       # (ain, aout) x 6
    attr_sh = di("attr_sh", [128, SB, F], BF16)
    mtb = di("mtb", [SB, 128, NB, 128], F8)   # M^T shard, block+partition major

    partials = nc.dram_tensor("partials", [4, 1], F32, kind="ExternalOutput")

    st = {
        "aT": {(1, LI_ENC0): a1n, (1, LI_ENC1): a1n,
               (1, LI_D10): a1p, (1, LI_D11): a1p,
               (2, LI_ENC0): a2n, (2, LI_ENC1): a2n,
               (2, LI_D20): a2p, (2, LI_D21): a2p},
        "attr_sh": attr_sh, "mtb": mtb, "partials": partials,
        "ar": {}, "ag": {}, "dbg": {},
    }
    warm_in = nc.dram_tensor("warm_in", [128, 8], F32)
    warm_out = nc.dram_tensor("warm_out", [128, 8], F32, addr_space="Shared")
    warm_out2 = nc.dram_tensor("warm_out2", [128, 8], F32, addr_space="Shared")
    warm_out3 = nc.dram_tensor("warm_out3", [128, 8], F32, addr_space="Shared")
    st["warm2"] = (warm_in, warm_out2)
    st["warm3"] = (warm_in, warm_out3)
    for c in (1, 2):
        for l in range(4):
            ai = nc.dram_tensor(f"ar_in_{c}_{l}", [128, 8], F32)
            ao = nc.dram_tensor(f"ar_out_{c}_{l}", [128, 8], F32,
                                addr_space="Shared")
            st["ar"][(c, l)] = (ai, ao)
            if l < 3:
                gi = nc.dram_tensor(f"ag_in_{c}_{l}", [128, SB * F], F8)
                go = nc.dram_tensor(f"ag_out_{c}_{l}", [128 * NCORES, SB * F],
                                    F8, addr_space="Shared")
                st["ag"][(c, l)] = (gi, go)
    st["ag2"] = (nc.dram_tensor("ag2_in", [128, SB * F], F8),
                 nc.dram_tensor("ag2_out", [128 * NCORES, SB * F], F8,
                                addr_space="Shared"))
    st["arg_in"] = nc.dram_tensor("arg_in", [128, FB * F], F32)
    st["arg_out"] = nc.dram_tensor("arg_out", [128 // NCORES, FB * F], F32)

    if os.environ.get("BASSK_DEBUG"):
        for c in (1, 2):
            for l in range(3):
                st["dbg"][(c, l)] = nc.dram_tensor(
                    f"dbg_h_{c}_{l}", [SH, F], F8, kind="ExternalOutput")
        st["dbg"]["h2n"] = nc.dram_tensor("dbg_h2n", [SH, F], F8,
                                          kind="ExternalOutput")

    LI = {1: [LI_ENC0, LI_ENC1, LI_D10, LI_D11],
          2: [LI_ENC0, LI_ENC1, LI_D20, LI_D21]}

    with tile.TileContext(nc) as tc:
        with (
            tc.tile_pool(name="sb", bufs=2) as sb,
            tc.tile_pool(name="ps", bufs=8, space="PSUM") as ps,
        ):
            # ---- constants / params ----
            g = {"w_all": w_all}
            ident = sb.tile([128, 128], BF16, tag="ident", name="ident")
            make_identity(nc, ident[:])
            g["ident"] = ident
            for nm, src in (("b_sb", b_all), ("g_sb", g_all), ("bb_sb", bb_all)):
                t = sb.tile([128, 6, FB], F32, tag=nm)
                nc.sync.dma_start(t[:], src.rearrange("l (m p) -> p l m", p=128))
                g[nm] = t
            al1 = sb.tile([1, 12], F32, tag="al1", name="al1")
            nc.sync.dma_start(al1[:], al_all[:])
            al_sb = sb.tile([128, 12], F32, tag="al_sb", name="al_sb")
            nc.gpsimd.partition_broadcast(al_sb[:], al1[:])
            g["al_sb"] = al_sb
            epsb = sb.tile([128, 1], F32, tag="epsb", name="epsb")
            nc.vector.memset(epsb[:], 1e-5)
            g["epsb"] = epsb

            # warm up the collective engine with a dummy AllReduce
            warm_sb = sb.tile([128, 8], F32, tag="warm", name="warm")
            nc.vector.memset(warm_sb[:], 0.0)
            nc.sync.dma_start(warm_in[:], warm_sb[:])
            nc.gpsimd.collective_compute(
                "AllReduce", ALU.add, replica_groups=[list(range(NCORES))],
                ins=[warm_in[:]], outs=[warm_out[:]])
            nc.gpsimd.collective_compute(
                "AllReduce", ALU.add, replica_groups=[list(range(NCORES))],
                ins=[st["warm2"][0][:]], outs=[st["warm2"][1][:]])
            nc.gpsimd.collective_compute(
                "AllReduce", ALU.add, replica_groups=[list(range(NCORES))],
                ins=[st["warm3"][0][:]], outs=[st["warm3"][1][:]])

            # initial h loads (fp8, partition-major)
            st["hsb"] = {}
            st["mt"] = {}
            st["zt"] = {}
            for c, h0 in ((1, h1_0), (2, h2_0)):
                hh = NB // 2
                ha = sb.tile([128, hh, F], F8, tag=f"h{c}a", bufs=1, name="ha")
                hb = sb.tile([128, hh, F], F8, tag=f"h{c}b", bufs=1, name="hb")
                nc.sync.dma_start(ha[:], h0[:, 0:hh, :])
                nc.sync.dma_start(hb[:], h0[:, hh:NB, :])
                st["hsb"][c] = (ha, hb)
                st["mt"][c] = sb.tile([128, FB, SH], F8, tag=f"mt{c}",
                                      bufs=1, name="mt")
                st["zt"][c] = sb.tile([128, FB, SH], BF16, tag=f"zt{c}",
                                      bufs=1, name="zt")

            # ---- pipelined rounds ----
            # Round layout (steady state), chosen so the PE work after each
            # tail's AllGather trigger (2 A-n0 chunks + all of A-n1 + W)
            # covers the AG + h-load latency:
            #   [A1-n0 c0-5][T2(l-1)][A1-n0 c6-7 + A1-n1 + W1 + AR1]
            #   [A2-n0 c0-5][T1(l)  ][A2-n0 c6-7 + A2-n1 + W2 + AR2]
            st["m2c"] = sb.tile([128, SB * (NB // MCK) * 4], F32,
                                tag="m2c", bufs=1, name="m2c")
            NCH = NB // ACK
            K1 = 4

            def art_hoist(c, l, chunks):
                # chunks: list of (n, cc)
                def h():
                    return [_art_dma(nc, sb, st, c, LI[c][l], n, cc)
                            for n, cc in chunks]
                return h

            def emit_M_n0(c, l, pre=(), nchunks=K1):
                mps = [ps.tile([128, 512], F32, tag="mmps", bufs=4, name="mmps")
                       for _ in range(FB)]
                li = LI[c][l]
                for cc in range(nchunks):
                    art = pre[cc] if cc < len(pre) else _art_dma(
                        nc, sb, st, c, li, 0, cc)
                    _emit_A_chunk(nc, st, c, mps, art, cc, 0)
                return mps

            def emit_M_tail(c, l, mps0, k1, pre=(), hoist=None):
                # finish n0 chunks k1.., then evict/W/stats/AR via M_rest
                li = LI[c][l]
                pre = list(pre)
                for cc in range(k1, NCH):
                    art = pre.pop(0) if pre else _art_dma(
                        nc, sb, st, c, li, 0, cc)
                    _emit_A_chunk(nc, st, c, mps0, art, cc, 0)
                return _emit_M_rest(nc, sb, ps, g, st, c, l, li, mps0,
                                    pre=tuple(pre), hoist=hoist)

            pre_a = ()
            for l in range(4):
                last = l == 3
                k1a = NCH if l == 0 else K1
                mps0 = emit_M_n0(1, l, pre=pre_a, nchunks=k1a)
                pre_c = ()
                if l > 0:
                    # hoist: chain-1's remaining n0 chunks + first n1 chunk
                    pre_c = _emit_T(nc, sb, ps, g, st, 2, l - 1, LI[2][l - 1],
                                    hoist=art_hoist(1, l, [(0, K1), (0, K1 + 1), (0, K1 + 2),
                                                           (0, K1 + 3), (1, 0)])) or ()
                    if l in (2, 3):
                        _emit_m2_pass(nc, sb, st,
                                      range(4 * (l - 2), 4 * (l - 1)))
                pre_d = emit_M_tail(1, l, mps0, k1a, pre=pre_c,
                                    hoist=art_hoist(2, l, [(0, 0), (0, 1)]))
                k1b = NCH if last else K1
                mps0b = emit_M_n0(2, l, pre=pre_d or (), nchunks=k1b)
                if not last:
                    pre_f = _emit_T(nc, sb, ps, g, st, 1, l, LI[1][l],
                                    hoist=art_hoist(2, l, [(0, K1), (0, K1 + 1), (0, K1 + 2),
                                                           (0, K1 + 3), (1, 0)])) or ()
                    hoist_a = art_hoist(1, l + 1, [(0, 0), (0, 1)])
                    pre_a = emit_M_tail(2, l, mps0b, K1, pre=pre_f,
                                        hoist=hoist_a) or ()
                else:
                    emit_M_tail(2, l, mps0b, NCH)
                    _emit_tail2_head(nc, sb, ps, g, st)
                    _emit_tail1(nc, sb, ps, g, st)
                    _emit_tail2_body(nc, sb, ps, g, st)

    nc.compile()
    return nc


_NC_CACHE = None


def _get_nc():
    global _NC_CACHE
    if _NC_CACHE is None:
        _NC_CACHE = build_nc()
    return _NC_CACHE


def _dinv(idx):
    deg = np.bincount(idx, minlength=N).astype(np.float32)
    return 1.0 / np.sqrt(np.clip(deg, 1.0, None))


def _adj_t(src, dst):
    """A^T[s, d] = multiplicity of edge s->d, float32 [N, N]."""
    flat = src.astype(np.int64) * N + dst.astype(np.int64)
    return np.bincount(flat, minlength=N * N).astype(np.float32).reshape(N, N)


def _pmaj(x):
    """[N, F'] -> [128, N/128, F'] partition-major."""
    n, f = x.shape
    return np.ascontiguousarray(x.reshape(n // 128, 128, f).transpose(1, 0, 2))


def _a_shard(at, c):
    """A^T column shard for core c as [2, 128, NB, 512] fp8."""
    sl = at[:, c * SH:(c + 1) * SH]
    halves = [_pmaj(np.ascontiguousarray(sl[:, n * 512:(n + 1) * 512]))
              for n in range(2)]
    return np.stack(halves).astype(ml_dtypes.float8_e4m3)


def host_prep(inputs):
    f8 = ml_dtypes.float8_e4m3
    bf16 = ml_dtypes.bfloat16
    attr = np.asarray(inputs["attr"], np.float32)
    matrix = np.asarray(inputs["matrix"], np.float32)
    mask1 = np.asarray(inputs["enc_mask_token1"], np.float32)
    src = np.asarray(inputs["src"]); dst = np.asarray(inputs["dst"])
    src2 = np.asarray(inputs["src2"]); dst2 = np.asarray(inputs["dst2"])
    tok = np.asarray(inputs["token_nodes"])
    noi = np.asarray(inputs["noise_nodes"])
    nsrc = np.asarray(inputs["noise_src"])

    x = attr.copy()
    x[tok] = 0.0
    x[noi] = attr[nsrc]
    np.add.at(x, tok, mask1[0])

    d1s, d1d = _dinv(src), _dinv(dst)
    d2s, d2d = _dinv(src2), _dinv(dst2)

    a1t = _adj_t(src, dst)
    a2t = _adj_t(src2, dst2)
    a1n = d1s[:, None] * a1t * d1d[None, :]
    a2n = d2s[:, None] * a2t * d2d[None, :]

    w_all = np.stack([
        np.asarray(inputs["enc_W"][0]), np.asarray(inputs["enc_W"][1]),
        np.asarray(inputs["dec1_W"][0]), np.asarray(inputs["dec1_W"][1]),
        np.asarray(inputs["dec2_W"][0]), np.asarray(inputs["dec2_W"][1]),
    ]).astype(f8)

    def stack6(key):
        return np.stack([
            np.asarray(inputs[f"enc_{key}"][0]), np.asarray(inputs[f"enc_{key}"][1]),
            np.asarray(inputs[f"dec1_{key}"][0]), np.asarray(inputs[f"dec1_{key}"][1]),
            np.asarray(inputs[f"dec2_{key}"][0]), np.asarray(inputs[f"dec2_{key}"][1]),
        ]).astype(np.float32)

    b_all, g_all, bb_all = stack6("b"), stack6("g"), stack6("bb")
    al = np.zeros((1, 12), np.float32)
    for i, (sa, so) in enumerate((("enc", 0), ("enc", 1), ("dec1", 0),
                                  ("dec1", 1), ("dec2", 0), ("dec2", 1))):
        al[0, 2 * i] = np.asarray(inputs[f"{sa}_ain"])[so]
        al[0, 2 * i + 1] = np.asarray(inputs[f"{sa}_aout"])[so]

    x_f8 = _pmaj(x).astype(f8)
    attr_f8 = _pmaj(attr).astype(f8)

    in_maps = []
    for c in range(NCORES):
        sl = slice(c * SH, (c + 1) * SH)
        mt_sh = np.ascontiguousarray(matrix[sl].T)          # [N, SH]
        mtb = np.stack([
            _pmaj(np.ascontiguousarray(mt_sh[:, b * 128:(b + 1) * 128]))
            for b in range(SB)
        ]).astype(f8)                                       # [SB, 128, NB, 128]
        attr_c = attr[sl].reshape(SB, 128, F).transpose(1, 0, 2)
        in_maps.append({
            "h1_0": x_f8, "h2_0": attr_f8,
            "a1n": _a_shard(a1n, c), "a1p": _a_shard(a1t, c),
            "a2n": _a_shard(a2n, c), "a2p": _a_shard(a2t, c),
            "w_all": w_all, "b_all": b_all, "g_all": g_all, "bb_all": bb_all,
            "al_all": al,
            "attr_sh": np.ascontiguousarray(attr_c).astype(bf16),
            "mtb": mtb,
        })
    return in_maps


def combine(results):
    l1 = sum(float(r["partials"][0, 0]) for r in results)
    dot = sum(float(r["partials"][1, 0]) for r in results)
    m2 = sum(float(r["partials"][2, 0]) for r in results)
    g2 = sum(float(r["partials"][3, 0]) for r in results)
    loss2 = (g2 - 2.0 * dot + m2) / (float(N) * N)
    loss = 0.5 * (l1 / N) + 0.5 * loss2
    return np.asarray(loss, dtype=np.float32)


def run(inputs, trace=False, trace_kwargs=None):
    nc = _get_nc()
    in_maps = host_prep(inputs)
    res = run_bass_kernel_spmd(nc, in_maps, core_ids=list(range(NCORES)),
                               trace=trace, **(trace_kwargs or {}))
    return combine(res.results), res


def kernel(**inputs) -> np.ndarray:
    out, _ = run(inputs, trace=False)
    return out
